# revision 22
# baseline (speedup 1.0000x reference)
"""AGNNConv (4 layers) on 8 Trainium2 NeuronCores — dense-attention,
chunk-overlapped collective formulation.

The runtime cannot execute data-dependent DMA, so the sparse GNN is
reformulated densely with a dst-shard: core c owns dst nodes
[1250c, 1250(c+1)).  M^T[v, d] (edge multiplicity) is a host-built dense
fp8 matrix per core, streamed from DRAM.  Per layer:

  P^T[v, d] = M^T[v, d] * exp(beta * cos(v, d))
  num[d, :] = sum_v P^T[v, d] * h_raw[v, :]     (PE, P^T stationary)
  denom[d]  = sum_v P^T[v, d] * 1               (appended ones column)
  h_next[d] = relu(num / (denom + eps))

Key design points vs the earlier dense baseline (1.96 ms):
  - BOTH matmul phases use fp8e4 DoubleRow (2x PE rate): phase 1
    contracts feature-tile pairs of the normalized state (entries in
    [-1, 1]); phase 2 contracts v-tile pairs with P^T and the raw rows
    quantized to e4m3 (rel err ~7e-3 vs the 2e-2 gate).
  - Node state is shared between layers by 5 CHUNKED AllGathers per
    boundary (one per 256-dst-row block), each issued as soon as its
    block's epilogue completes, so the collective overlaps the remaining
    dblocks' compute instead of serializing between layers.
  - The AG payload is halved: only [hn | norm] rows (257 bf16 cols) are
    shipped.  Receivers rebuild h_raw rows in place with one per-tile
    DVE scale (h_raw = hn * norm, then ones-column memset) and rebuild
    the transposed feature-major copy with dma_start_transpose (XBAR)
    plus a DVE bf16->f8 cast — no PE/ACT cost.
  - ACT does only Exp (the softmax numerators are its ~83us/layer
    floor) plus two tiny Ln/Exp ops per dst tile: 1/sqrt is computed as
    Exp(-0.5 * Ln(ssq + eps)), and the epilogue's relu/square/scale run
    on DVE.  A single ACT table load serves the whole kernel: the
    act-table placement pass resolves each function to the first set
    containing it (thrashing exp_and_others <-> natural_log ~2.7us per
    reload), so _build hides Exp/Ln from every set except the combined
    natural_log_exp_and_others before compiling.
  - Padded global node order is chunk-major: v' = 2048k + 256c + r for
    local dst row kr = 256k + r of core c, so AllGather output chunks are
    contiguous v' ranges and every padded row is some core's computed
    dst row (pad rows stage finite junk that M^T = 0 annihilates).

`repeats` executes the whole 4-layer kernel R times in one dispatch
(reloading the initial state each repeat) so device time can be
measured differentially without per-dispatch overhead (~84 ms under
axon): HW time ~= (wall_R5 - wall_R1) / 4.
"""

import numpy as np
import ml_dtypes

BF16 = ml_dtypes.bfloat16
F8 = ml_dtypes.float8_e4m3fn

N_NODES = 10000
D = 256
N_LAYERS = 4
EPS = 1e-16
NCORES = 8
PER = N_NODES // NCORES          # 1250 dst nodes per core
NCHUNK = 5                       # AG chunks per layer boundary
CROWS = 256                      # dst rows per chunk (2 dst tiles)
SH = NCHUNK * CROWS              # 1280 padded local rows per core
NP = SH * NCORES                 # 10240 padded global rows
VT = NP // 128                   # 80 v-tiles
DB = 256                         # dst-block width (= CROWS)
RCOL = D + 1                     # [hn | norm]
GROW = NCORES * CROWS            # 2048 global rows per AG-out chunk

_BUILD_CACHE = {}


def _row_of(n):
    ln = n % PER
    return (ln // CROWS) * GROW + (n // PER) * CROWS + ln % CROWS


def _build(n_layers=N_LAYERS, repeats=1, model_mode=False):
    import concourse.mybir as mybir
    import concourse.tile as tile
    from concourse import bacc

    f32 = mybir.dt.float32
    bf16 = mybir.dt.bfloat16
    f8 = mybir.dt.float8e4
    AF = mybir.ActivationFunctionType
    OP = mybir.AluOpType
    DR = mybir.MatmulPerfMode.DoubleRow

    nc = bacc.Bacc("TRN2", target_bir_lowering=False, debug=False,
                   num_devices=NCORES)
    # Steer the act-table-load pass: Exp/Ln each appear in several table
    # sets and the pass resolves each to the first containing set, which
    # makes Exp<->Ln interleavings thrash exp_and_others <-> natural_log
    # (~2.7us per reload).  Hiding them from every set except the combined
    # one makes the whole kernel run off a single table load.  This only
    # affects load *placement*; the real hardware table set genuinely
    # contains both functions.
    tabs = bacc.get_activation_tables(nc.m.arch)
    for name, fns in tabs.items():
        if name != "natural_log_exp_and_others":
            fns.discard(AF.Exp)
            fns.discard(AF.Ln)
    # host-prelaid SBUF-layout inputs (one contiguous DMA each)
    hT0_d = nc.dram_tensor("hT0", [128, VT * 2 * 128], f8,
                           kind="ExternalInput")
    hD0_d = nc.dram_tensor("hD0", [128, NCHUNK * 2 * DB], f8,
                           kind="ExternalInput")
    hr0_d = nc.dram_tensor("hr0", [128, VT * RCOL], f8,
                           kind="ExternalInput")
    mt_d = nc.dram_tensor("mt", [128, NCHUNK * 20 * 4 * DB], f8,
                          kind="ExternalInput")
    ident_d = nc.dram_tensor("ident", [128, 128], bf16, kind="ExternalInput")
    betas_d = nc.dram_tensor("betas", [128, N_LAYERS], f32,
                             kind="ExternalInput")
    out_d = nc.dram_tensor("out", [PER, D], f32, kind="ExternalOutput")

    total = n_layers * repeats

    with tile.TileContext(nc) as tc:
        with (
            tc.tile_pool(name="res", bufs=1) as res_pool,
            tc.tile_pool(name="pt", bufs=2) as pt_pool,
            tc.tile_pool(name="stream", bufs=6) as st_pool,
            tc.tile_pool(name="tT", bufs=3) as tT_pool,
            tc.tile_pool(name="small", bufs=2) as small_pool,
            tc.tile_pool(name="ps1", bufs=2, space="PSUM") as ps1_pool,
            tc.tile_pool(name="ps2", bufs=2, space="PSUM") as ps2_pool,
            tc.tile_pool(name="pst", bufs=2, space="PSUM") as pst_pool,
            tc.tile_pool(name="dram", bufs=1, space="DRAM") as dram_pool,
        ):
            # ------- resident SBUF state -------
            hnTb = [res_pool.tile([128, VT, 2, 128], f8, name=f"hnT{i}")
                    for i in range(2)]
            hnDb = [res_pool.tile([128, NCHUNK, 2, DB], f8, name=f"hnD{i}")
                    for i in range(2)]
            hr = res_pool.tile([128, VT, RCOL], f8, name="hr")
            ident = res_pool.tile([128, 128], bf16, name="ident")
            betas = res_pool.tile([128, N_LAYERS], f32, name="betas")
            nc.sync.dma_start(ident[:], ident_d.ap()[:])
            nc.sync.dma_start(betas[:], betas_d.ap()[:])

            agI, agO = {}, {}
            for l in range(total):
                if l % n_layers == n_layers - 1 or l == total - 1:
                    continue
                for k in range(NCHUNK):
                    agI[(l + 1, k)] = dram_pool.tile(
                        [CROWS, RCOL], bf16, name=f"agI{l + 1}_{k}")
                    agO[(l + 1, k)] = dram_pool.tile(
                        [GROW, RCOL], bf16, addr_space="Shared",
                        name=f"agO{l + 1}_{k}")

            hT0_v = hT0_d.ap().rearrange("p (a k j) -> p a k j", k=2, j=128)
            hD0_v = hD0_d.ap().rearrange("p (b k j) -> p b k j", k=2, j=DB)
            hr0_v = hr0_d.ap().rearrange("p (t f) -> p t f", f=RCOL)
            # 8-tile mask slabs: 2KB/partition per DMA halves the trigger
            # count + descriptor load on the SP sequencer vs 4-tile fetches
            mt_v = mt_d.ap().rearrange("p (b s q j) -> p b s q j",
                                       b=NCHUNK, s=10, q=8)

            for l in range(total):
                lab = l % n_layers
                last_of_rep = lab == n_layers - 1
                hnT = hnTb[l % 2]
                hnT_nxt = hnTb[(l + 1) % 2]
                hnD = hnDb[l % 2]
                hnD_nxt = hnDb[(l + 1) % 2]
                beta_col = betas[:, lab:lab + 1]

                if lab == 0:
                    nc.sync.dma_start(hnT[:], hT0_v)
                    nc.sync.dma_start(hnD[:], hD0_v)
                    nc.sync.dma_start(hr[:], hr0_v)

                for nb in range(NCHUNK):
                    ptb = pt_pool.tile([128, VT, DB], f8, tag="ptb",
                                       name="ptb")
                    # ---- phase 1: P^T[v, nb*256 : +256] ----
                    mt_t = None
                    for ap0 in range(0, VT, 4):
                        ps = ps1_pool.tile([128, 4, DB], f32, tag="ps1",
                                           name="ps")
                        for g in range(4):
                            a = ap0 + g
                            nc.tensor.matmul(
                                ps[:, g, :], hnT[:, a, :, :],
                                hnD[:, nb, :, :], perf_mode=DR,
                                start=True, stop=True)
                        if ap0 % 8 == 0:
                            mt_t = st_pool.tile([128, 8, DB], f8, tag="mt",
                                                name="mt_t")
                            nc.sync.dma_start(mt_t[:], mt_v[:, nb, ap0 // 8])
                        mh = (ap0 // 4) % 2
                        nc.scalar.activation(
                            ptb[:, ap0:ap0 + 4, :], ps[:],
                            AF.Exp, scale=beta_col)
                        nc.vector.tensor_tensor(
                            ptb[:, ap0:ap0 + 4, :], ptb[:, ap0:ap0 + 4, :],
                            mt_t[:, mh * 4:mh * 4 + 4, :], OP.mult)

                    # ---- phase 2 + epilogue per 128-dst tile ----
                    for half in range(2):
                        gd0 = nb * DB + half * 128
                        ps2 = ps2_pool.tile([128, RCOL], f32, tag="ps2",
                                            name="ps2")
                        for a in range(0, VT, 2):
                            nc.tensor.matmul(
                                ps2[:],
                                ptb[:, a:a + 2, half * 128:half * 128 + 128],
                                hr[:, a:a + 2, :], perf_mode=DR,
                                start=(a == 0), stop=(a == VT - 2))
                        deps = small_pool.tile([128, 1], f32, tag="deps",
                                               name="deps")
                        nc.vector.tensor_scalar_add(deps[:], ps2[:, D:D + 1],
                                                    EPS)
                        invd = small_pool.tile([128, 1], f32, tag="invd",
                                               name="invd")
                        nc.vector.reciprocal(invd[:], deps[:])
                        hraw = small_pool.tile([128, D], f32, tag="hraw",
                                               name="hraw")
                        nc.vector.tensor_scalar(hraw[:], ps2[:, 0:D],
                                                invd[:], 0.0, OP.mult,
                                                OP.max)
                        if last_of_rep:
                            dt = min(128, PER - gd0)
                            nc.sync.dma_start(
                                out_d.ap()[gd0:gd0 + dt, :], hraw[0:dt, :])
                            continue
                        sq = small_pool.tile([128, D], bf16, tag="sq",
                                             name="sq")
                        ssq = small_pool.tile([128, 1], f32, tag="ssq",
                                              name="ssq")
                        nc.vector.scalar_tensor_tensor(
                            sq[:], hraw[:], 1.0, hraw[:], OP.mult, OP.mult,
                            accum_out=ssq[:])
                        sse = small_pool.tile([128, 1], f32, tag="sse",
                                              name="sse")
                        nc.vector.tensor_scalar_add(sse[:], ssq[:], EPS)
                        lnv = small_pool.tile([128, 1], f32, tag="lnv",
                                              name="lnv")
                        nc.scalar.activation(lnv[:], sse[:], AF.Ln)
                        invn = small_pool.tile([128, 1], f32, tag="invn",
                                               name="invn")
                        nc.scalar.activation(invn[:], lnv[:], AF.Exp,
                                             scale=-0.5)
                        norm = small_pool.tile([128, 1], f32, tag="norm",
                                               name="norm")
                        nc.vector.tensor_mul(norm[:], sse[:], invn[:])
                        stg = small_pool.tile([128, RCOL], bf16, tag="stg",
                                              name="stg")
                        nc.vector.tensor_scalar_mul(stg[:, 0:D], hraw[:],
                                                    invn[:])
                        nc.vector.tensor_copy(stg[:, D:D + 1], norm[:])
                        nc.sync.dma_start(
                            agI[(l + 1, nb)][half * 128:half * 128 + 128, :],
                            stg[:])
                        # local transposed normalized copy for next phase-1
                        for kf in range(2):
                            pst = pst_pool.tile([128, 128], bf16, tag="pst",
                                                name="pst")
                            nc.tensor.transpose(
                                pst[:], stg[:, kf * 128:(kf + 1) * 128],
                                ident[:])
                            nc.vector.tensor_copy(
                                hnD_nxt[:, nb, kf,
                                        half * 128:half * 128 + 128],
                                pst[:])

                    if last_of_rep:
                        continue
                    # ---- chunk nb staged: AllGather + transposed rebuild
                    # (hnT_nxt is the pong buffer, safe to fill mid-layer) ----
                    if not model_mode:
                        nc.gpsimd.collective_compute(
                            "AllGather", mybir.AluOpType.bypass,
                            replica_groups=[list(range(NCORES))],
                            ins=[agI[(l + 1, nb)].opt()],
                            outs=[agO[(l + 1, nb)].opt()])
                    agOv = agO[(l + 1, nb)]
                    T0 = 16 * nb
                    for kf in range(2):
                        tT = tT_pool.tile([128, GROW], bf16, tag="tT",
                                          name="tT")
                        nc.sync.dma_start_transpose(
                            tT[:], agOv[:, kf * 128:(kf + 1) * 128])
                        nc.vector.tensor_copy(
                            hnT_nxt[:, T0:T0 + 16, kf, :],
                            tT[:].rearrange("p (t j) -> p t j", j=128))

                # ---- hr rebuild for the next layer: must wait until the
                # last dblock's phase-2 (every dblock reads ALL of hr), so
                # issue all 5 chunk row-loads + in-place scales here ----
                if last_of_rep:
                    continue
                for nb in range(NCHUNK):
                    agOv = agO[(l + 1, nb)]
                    T0 = 16 * nb
                    hstg = tT_pool.tile([128, 16, RCOL], bf16, tag="hstg",
                                        name="hstg")
                    nc.sync.dma_start(
                        hstg[:], agOv.rearrange("(t p) f -> p t f", p=128))
                    normf = small_pool.tile([128, 16, 1], f32, tag="normf",
                                            name="normf")
                    nc.vector.tensor_copy(normf[:], hstg[:, :, D:D + 1])
                    for t in range(16):
                        nc.vector.tensor_scalar_mul(
                            hr[:, T0 + t, 0:D], hstg[:, t, 0:D],
                            normf[:, t, :])
                    nc.vector.memset(hr[:, T0:T0 + 16, D:D + 1], 1.0)

    nc.compile()
    return nc


def _get_kernel():
    if "nc" not in _BUILD_CACHE:
        _BUILD_CACHE["nc"] = _build()
    return _BUILD_CACHE["nc"]


def _run(nc, in_maps, trace=False):
    from concourse.bass_utils import run_bass_kernel_spmd
    return run_bass_kernel_spmd(nc, in_maps, core_ids=list(range(NCORES)),
                                trace=trace)


def _make_in_maps(feats, src, dst, betas):
    feats = np.asarray(feats, dtype=np.float32)
    src = np.asarray(src, dtype=np.int64)
    dst = np.asarray(dst, dtype=np.int64)

    rows = _row_of(np.arange(N_NODES))
    ss = np.sum(feats * feats, axis=-1)
    invn = (1.0 / np.sqrt(ss + EPS)).astype(np.float32)
    hn = feats * invn[:, None]

    # transposed normalized features, SBUF layout [128, VT, 2, 128] f8
    hng = np.zeros((NP, D), dtype=np.float32)
    hng[rows] = hn
    hT0 = (hng.T.reshape(2, 128, VT, 128).transpose(1, 2, 0, 3)
           .reshape(128, VT * 2 * 128)).astype(F8)
    hT0 = np.ascontiguousarray(hT0)

    # raw rows [h | 1], SBUF layout [128, VT, RCOL] bf16
    hrg = np.zeros((NP, RCOL), dtype=np.float32)
    hrg[rows, 0:D] = feats
    hrg[rows, D] = 1.0
    hr0 = (hrg.reshape(VT, 128, RCOL).transpose(1, 0, 2)
           .reshape(128, VT * RCOL)).astype(F8)
    hr0 = np.ascontiguousarray(hr0)

    # per-core: own-shard transposed normalized [128, NCHUNK, 2, DB] f8
    # and the mask in streaming layout [128, NCHUNK, 20, 4, DB] f8
    vrow = rows[src]
    core = dst // PER
    dloc = dst % PER
    ident = np.eye(128, dtype=np.float32).astype(BF16)
    betas_rep = np.ascontiguousarray(
        np.tile(np.asarray(betas, dtype=np.float32)[None, :], (128, 1)))

    in_maps = []
    for c in range(NCORES):
        own = np.zeros((SH, D), dtype=np.float32)
        own[0:PER] = hn[c * PER:(c + 1) * PER]
        hD0 = (own.T.reshape(2, 128, NCHUNK, DB).transpose(1, 2, 0, 3)
               .reshape(128, NCHUNK * 2 * DB)).astype(F8)

        sel = core == c
        mt = np.zeros((NP, SH), dtype=np.float32)
        np.add.at(mt, (vrow[sel], dloc[sel]), 1.0)
        assert mt.max() <= 16, "edge multiplicity exceeds exact f8 ints"
        mtl = (mt.reshape(VT, 128, NCHUNK, DB).transpose(1, 2, 0, 3)
               .reshape(128, NCHUNK, 20, 4, DB)
               .reshape(128, NCHUNK * 20 * 4 * DB)).astype(F8)

        in_maps.append(dict(
            hT0=hT0,
            hD0=np.ascontiguousarray(hD0),
            hr0=hr0,
            mt=np.ascontiguousarray(mtl),
            ident=ident,
            betas=betas_rep,
        ))
    return in_maps


def kernel(feats, src, dst, betas):
    import time

    in_maps = _make_in_maps(feats, src, dst, betas)
    nc = _get_kernel()
    res = None
    last_err = None
    for attempt in range(3):
        try:
            res = _run(nc, in_maps)
            break
        except Exception as e:  # transient device/tunnel hiccups observed
            last_err = e
            time.sleep(20)
    if res is None:
        raise last_err
    shards = [np.asarray(res.results[c]["out"], dtype=np.float32)
              for c in range(NCORES)]
    return np.concatenate(shards, axis=0)


# revision 23
# speedup vs baseline: 1.0696x; 1.0696x over previous
"""AGNNConv (4 layers) on 8 Trainium2 NeuronCores — dense-attention,
chunk-overlapped collective formulation.

The runtime cannot execute data-dependent DMA, so the sparse GNN is
reformulated densely with a dst-shard: core c owns dst nodes
[1250c, 1250(c+1)).  M^T[v, d] (edge multiplicity) is a host-built dense
fp8 matrix per core, streamed from DRAM.  Per layer:

  P^T[v, d] = M^T[v, d] * exp(beta * cos(v, d))
  num[d, :] = sum_v P^T[v, d] * h_raw[v, :]     (PE, P^T stationary)
  denom[d]  = sum_v P^T[v, d] * 1               (appended ones column)
  h_next[d] = relu(num / (denom + eps))

Key design points vs the earlier dense baseline (1.96 ms):
  - BOTH matmul phases use fp8e4 DoubleRow (2x PE rate): phase 1
    contracts feature-tile pairs of the normalized state (entries in
    [-1, 1]); phase 2 contracts v-tile pairs with P^T and the raw rows
    quantized to e4m3 (rel err ~7e-3 vs the 2e-2 gate).
  - Node state is shared between layers by 5 CHUNKED AllGathers per
    boundary (one per 256-dst-row block), each issued as soon as its
    block's epilogue completes, so the collective overlaps the remaining
    dblocks' compute instead of serializing between layers.
  - The AG payload is halved: only [hn | norm] rows (257 bf16 cols) are
    shipped.  Receivers rebuild h_raw rows in place with one per-tile
    DVE scale (h_raw = hn * norm, then ones-column memset) and rebuild
    the transposed feature-major copy with dma_start_transpose (XBAR)
    plus a DVE bf16->f8 cast — no PE/ACT cost.
  - ACT does only Exp (the softmax numerators are its ~83us/layer
    floor) plus two tiny Ln/Exp ops per dst tile: 1/sqrt is computed as
    Exp(-0.5 * Ln(ssq + eps)), and the epilogue's relu/square/scale run
    on DVE.  A single ACT table load serves the whole kernel: the
    act-table placement pass resolves each function to the first set
    containing it (thrashing exp_and_others <-> natural_log ~2.7us per
    reload), so _build hides Exp/Ln from every set except the combined
    natural_log_exp_and_others before compiling.
  - Padded global node order is chunk-major: v' = 2048k + 256c + r for
    local dst row kr = 256k + r of core c, so AllGather output chunks are
    contiguous v' ranges and every padded row is some core's computed
    dst row (pad rows stage finite junk that M^T = 0 annihilates).

`repeats` executes the whole 4-layer kernel R times in one dispatch
(reloading the initial state each repeat) so device time can be
measured differentially without per-dispatch overhead (~84 ms under
axon): HW time ~= (wall_R5 - wall_R1) / 4.
"""

import numpy as np
import ml_dtypes

BF16 = ml_dtypes.bfloat16
F8 = ml_dtypes.float8_e4m3fn

N_NODES = 10000
D = 256
N_LAYERS = 4
EPS = 1e-16
NCORES = 8
PER = N_NODES // NCORES          # 1250 dst nodes per core
NCHUNK = 5                       # AG chunks per layer boundary
CROWS = 256                      # dst rows per chunk (2 dst tiles)
SH = NCHUNK * CROWS              # 1280 padded local rows per core
NP = SH * NCORES                 # 10240 padded global rows
VT = NP // 128                   # 80 v-tiles
DB = 256                         # dst-block width (= CROWS)
RCOL = D + 1                     # [hn | norm]
GROW = NCORES * CROWS            # 2048 global rows per AG-out chunk

_BUILD_CACHE = {}


def _row_of(n):
    ln = n % PER
    return (ln // CROWS) * GROW + (n // PER) * CROWS + ln % CROWS


def _build(n_layers=N_LAYERS, repeats=1, model_mode=False):
    import concourse.mybir as mybir
    import concourse.tile as tile
    from concourse import bacc

    f32 = mybir.dt.float32
    bf16 = mybir.dt.bfloat16
    f8 = mybir.dt.float8e4
    AF = mybir.ActivationFunctionType
    OP = mybir.AluOpType
    DR = mybir.MatmulPerfMode.DoubleRow

    nc = bacc.Bacc("TRN2", target_bir_lowering=False, debug=False,
                   num_devices=NCORES)
    # Steer the act-table-load pass: Exp/Ln each appear in several table
    # sets and the pass resolves each to the first containing set, which
    # makes Exp<->Ln interleavings thrash exp_and_others <-> natural_log
    # (~2.7us per reload).  Hiding them from every set except the combined
    # one makes the whole kernel run off a single table load.  This only
    # affects load *placement*; the real hardware table set genuinely
    # contains both functions.
    tabs = bacc.get_activation_tables(nc.m.arch)
    for name, fns in tabs.items():
        if name != "natural_log_exp_and_others":
            fns.discard(AF.Exp)
            fns.discard(AF.Ln)
    # host-prelaid SBUF-layout inputs (one contiguous DMA each)
    hT0_d = nc.dram_tensor("hT0", [128, VT * 2 * 128], f8,
                           kind="ExternalInput")
    hD0_d = nc.dram_tensor("hD0", [128, NCHUNK * 2 * DB], f8,
                           kind="ExternalInput")
    hr0_d = nc.dram_tensor("hr0", [128, VT * RCOL], f8,
                           kind="ExternalInput")
    mt_d = nc.dram_tensor("mt", [128, NCHUNK * 20 * 4 * DB], f8,
                          kind="ExternalInput")
    ident_d = nc.dram_tensor("ident", [128, 128], bf16, kind="ExternalInput")
    betas_d = nc.dram_tensor("betas", [128, N_LAYERS], f32,
                             kind="ExternalInput")
    out_d = nc.dram_tensor("out", [PER, D], f32, kind="ExternalOutput")

    total = n_layers * repeats

    with tile.TileContext(nc) as tc:
        with (
            tc.tile_pool(name="res", bufs=1) as res_pool,
            tc.tile_pool(name="pt", bufs=2) as pt_pool,
            tc.tile_pool(name="stream", bufs=6) as st_pool,
            tc.tile_pool(name="tT", bufs=3) as tT_pool,
            tc.tile_pool(name="small", bufs=2) as small_pool,
            tc.tile_pool(name="ps1", bufs=2, space="PSUM") as ps1_pool,
            tc.tile_pool(name="ps2", bufs=2, space="PSUM") as ps2_pool,
            tc.tile_pool(name="pst", bufs=2, space="PSUM") as pst_pool,
            tc.tile_pool(name="dram", bufs=1, space="DRAM") as dram_pool,
        ):
            # ------- resident SBUF state -------
            hnTb = [res_pool.tile([128, VT, 2, 128], f8, name=f"hnT{i}")
                    for i in range(2)]
            hnDb = [res_pool.tile([128, NCHUNK, 2, DB], f8, name=f"hnD{i}")
                    for i in range(2)]
            hrb = [res_pool.tile([128, VT, RCOL], f8, name=f"hr{i}")
                   for i in range(2)]
            ident = res_pool.tile([128, 128], bf16, name="ident")
            betas = res_pool.tile([128, N_LAYERS], f32, name="betas")
            nc.sync.dma_start(ident[:], ident_d.ap()[:])
            nc.sync.dma_start(betas[:], betas_d.ap()[:])

            agI, agO = {}, {}
            for l in range(total):
                if l % n_layers == n_layers - 1 or l == total - 1:
                    continue
                for k in range(NCHUNK):
                    agI[(l + 1, k)] = dram_pool.tile(
                        [CROWS, RCOL], bf16, name=f"agI{l + 1}_{k}")
                    agO[(l + 1, k)] = dram_pool.tile(
                        [GROW, RCOL], bf16, addr_space="Shared",
                        name=f"agO{l + 1}_{k}")

            hT0_v = hT0_d.ap().rearrange("p (a k j) -> p a k j", k=2, j=128)
            hD0_v = hD0_d.ap().rearrange("p (b k j) -> p b k j", k=2, j=DB)
            hr0_v = hr0_d.ap().rearrange("p (t f) -> p t f", f=RCOL)
            # 8-tile mask slabs: 2KB/partition per DMA halves the trigger
            # count + descriptor load on the SP sequencer vs 4-tile fetches
            mt_v = mt_d.ap().rearrange("p (b s q j) -> p b s q j",
                                       b=NCHUNK, s=10, q=8)

            for l in range(total):
                lab = l % n_layers
                last_of_rep = lab == n_layers - 1
                hnT = hnTb[l % 2]
                hnT_nxt = hnTb[(l + 1) % 2]
                hnD = hnDb[l % 2]
                hnD_nxt = hnDb[(l + 1) % 2]
                hr = hrb[l % 2]
                hr_nxt = hrb[(l + 1) % 2]
                beta_col = betas[:, lab:lab + 1]

                if lab == 0:
                    nc.sync.dma_start(hnT[:], hT0_v)
                    nc.sync.dma_start(hnD[:], hD0_v)
                    nc.sync.dma_start(hr[:], hr0_v)

                for nb in range(NCHUNK):
                    ptb = pt_pool.tile([128, VT, DB], f8, tag="ptb",
                                       name="ptb")
                    # ---- phase 1: P^T[v, nb*256 : +256] ----
                    mt_t = None
                    for ap0 in range(0, VT, 4):
                        ps = ps1_pool.tile([128, 4, DB], f32, tag="ps1",
                                           name="ps")
                        for g in range(4):
                            a = ap0 + g
                            nc.tensor.matmul(
                                ps[:, g, :], hnT[:, a, :, :],
                                hnD[:, nb, :, :], perf_mode=DR,
                                start=True, stop=True)
                        if ap0 % 8 == 0:
                            mt_t = st_pool.tile([128, 8, DB], f8, tag="mt",
                                                name="mt_t")
                            nc.sync.dma_start(mt_t[:], mt_v[:, nb, ap0 // 8])
                        mh = (ap0 // 4) % 2
                        nc.scalar.activation(
                            ptb[:, ap0:ap0 + 4, :], ps[:],
                            AF.Exp, scale=beta_col)
                        nc.vector.tensor_tensor(
                            ptb[:, ap0:ap0 + 4, :], ptb[:, ap0:ap0 + 4, :],
                            mt_t[:, mh * 4:mh * 4 + 4, :], OP.mult)

                    # ---- phase 2 + epilogue per 128-dst tile ----
                    for half in range(2):
                        gd0 = nb * DB + half * 128
                        ps2 = ps2_pool.tile([128, RCOL], f32, tag="ps2",
                                            name="ps2")
                        for a in range(0, VT, 2):
                            nc.tensor.matmul(
                                ps2[:],
                                ptb[:, a:a + 2, half * 128:half * 128 + 128],
                                hr[:, a:a + 2, :], perf_mode=DR,
                                start=(a == 0), stop=(a == VT - 2))
                        deps = small_pool.tile([128, 1], f32, tag="deps",
                                               name="deps")
                        nc.vector.tensor_scalar_add(deps[:], ps2[:, D:D + 1],
                                                    EPS)
                        invd = small_pool.tile([128, 1], f32, tag="invd",
                                               name="invd")
                        nc.vector.reciprocal(invd[:], deps[:])
                        hraw = small_pool.tile([128, D], f32, tag="hraw",
                                               name="hraw")
                        nc.vector.tensor_scalar(hraw[:], ps2[:, 0:D],
                                                invd[:], 0.0, OP.mult,
                                                OP.max)
                        if last_of_rep:
                            dt = min(128, PER - gd0)
                            nc.sync.dma_start(
                                out_d.ap()[gd0:gd0 + dt, :], hraw[0:dt, :])
                            continue
                        sq = small_pool.tile([128, D], bf16, tag="sq",
                                             name="sq")
                        ssq = small_pool.tile([128, 1], f32, tag="ssq",
                                              name="ssq")
                        nc.vector.scalar_tensor_tensor(
                            sq[:], hraw[:], 1.0, hraw[:], OP.mult, OP.mult,
                            accum_out=ssq[:])
                        sse = small_pool.tile([128, 1], f32, tag="sse",
                                              name="sse")
                        nc.vector.tensor_scalar_add(sse[:], ssq[:], EPS)
                        lnv = small_pool.tile([128, 1], f32, tag="lnv",
                                              name="lnv")
                        nc.scalar.activation(lnv[:], sse[:], AF.Ln)
                        invn = small_pool.tile([128, 1], f32, tag="invn",
                                               name="invn")
                        nc.scalar.activation(invn[:], lnv[:], AF.Exp,
                                             scale=-0.5)
                        norm = small_pool.tile([128, 1], f32, tag="norm",
                                               name="norm")
                        nc.vector.tensor_mul(norm[:], sse[:], invn[:])
                        stg = small_pool.tile([128, RCOL], bf16, tag="stg",
                                              name="stg")
                        nc.vector.tensor_scalar_mul(stg[:, 0:D], hraw[:],
                                                    invn[:])
                        nc.vector.tensor_copy(stg[:, D:D + 1], norm[:])
                        nc.sync.dma_start(
                            agI[(l + 1, nb)][half * 128:half * 128 + 128, :],
                            stg[:])
                        # local transposed normalized copy for next phase-1
                        for kf in range(2):
                            pst = pst_pool.tile([128, 128], bf16, tag="pst",
                                                name="pst")
                            nc.tensor.transpose(
                                pst[:], stg[:, kf * 128:(kf + 1) * 128],
                                ident[:])
                            nc.vector.tensor_copy(
                                hnD_nxt[:, nb, kf,
                                        half * 128:half * 128 + 128],
                                pst[:])

                    if last_of_rep:
                        continue
                    # ---- chunk nb staged: AllGather + transposed rebuild
                    # (hnT_nxt is the pong buffer, safe to fill mid-layer) ----
                    if not model_mode:
                        nc.gpsimd.collective_compute(
                            "AllGather", mybir.AluOpType.bypass,
                            replica_groups=[list(range(NCORES))],
                            ins=[agI[(l + 1, nb)].opt()],
                            outs=[agO[(l + 1, nb)].opt()])
                    agOv = agO[(l + 1, nb)]
                    T0 = 16 * nb
                    for kf in range(2):
                        tT = tT_pool.tile([128, GROW], bf16, tag="tT",
                                          name="tT")
                        nc.sync.dma_start_transpose(
                            tT[:], agOv[:, kf * 128:(kf + 1) * 128])
                        nc.vector.tensor_copy(
                            hnT_nxt[:, T0:T0 + 16, kf, :],
                            tT[:].rearrange("p (t j) -> p t j", j=128))
                    # row rebuild straight into the pong state buffer —
                    # runs mid-layer, no WAR with this layer's reads
                    hstg = tT_pool.tile([128, 16, RCOL], bf16, tag="hstg",
                                        name="hstg")
                    nc.sync.dma_start(
                        hstg[:], agOv.rearrange("(t p) f -> p t f", p=128))
                    normf = small_pool.tile([128, 16, 1], f32, tag="normf",
                                            name="normf")
                    nc.vector.tensor_copy(normf[:], hstg[:, :, D:D + 1])
                    for t in range(16):
                        nc.vector.tensor_scalar_mul(
                            hr_nxt[:, T0 + t, 0:D], hstg[:, t, 0:D],
                            normf[:, t, :])
                    nc.vector.memset(hr_nxt[:, T0:T0 + 16, D:D + 1], 1.0)

    nc.compile()
    return nc


def _get_kernel():
    if "nc" not in _BUILD_CACHE:
        _BUILD_CACHE["nc"] = _build()
    return _BUILD_CACHE["nc"]


def _run(nc, in_maps, trace=False):
    from concourse.bass_utils import run_bass_kernel_spmd
    return run_bass_kernel_spmd(nc, in_maps, core_ids=list(range(NCORES)),
                                trace=trace)


def _make_in_maps(feats, src, dst, betas):
    feats = np.asarray(feats, dtype=np.float32)
    src = np.asarray(src, dtype=np.int64)
    dst = np.asarray(dst, dtype=np.int64)

    rows = _row_of(np.arange(N_NODES))
    ss = np.sum(feats * feats, axis=-1)
    invn = (1.0 / np.sqrt(ss + EPS)).astype(np.float32)
    hn = feats * invn[:, None]

    # transposed normalized features, SBUF layout [128, VT, 2, 128] f8
    hng = np.zeros((NP, D), dtype=np.float32)
    hng[rows] = hn
    hT0 = (hng.T.reshape(2, 128, VT, 128).transpose(1, 2, 0, 3)
           .reshape(128, VT * 2 * 128)).astype(F8)
    hT0 = np.ascontiguousarray(hT0)

    # raw rows [h | 1], SBUF layout [128, VT, RCOL] bf16
    hrg = np.zeros((NP, RCOL), dtype=np.float32)
    hrg[rows, 0:D] = feats
    hrg[rows, D] = 1.0
    hr0 = (hrg.reshape(VT, 128, RCOL).transpose(1, 0, 2)
           .reshape(128, VT * RCOL)).astype(F8)
    hr0 = np.ascontiguousarray(hr0)

    # per-core: own-shard transposed normalized [128, NCHUNK, 2, DB] f8
    # and the mask in streaming layout [128, NCHUNK, 20, 4, DB] f8
    vrow = rows[src]
    core = dst // PER
    dloc = dst % PER
    ident = np.eye(128, dtype=np.float32).astype(BF16)
    betas_rep = np.ascontiguousarray(
        np.tile(np.asarray(betas, dtype=np.float32)[None, :], (128, 1)))

    in_maps = []
    for c in range(NCORES):
        own = np.zeros((SH, D), dtype=np.float32)
        own[0:PER] = hn[c * PER:(c + 1) * PER]
        hD0 = (own.T.reshape(2, 128, NCHUNK, DB).transpose(1, 2, 0, 3)
               .reshape(128, NCHUNK * 2 * DB)).astype(F8)

        sel = core == c
        mt = np.zeros((NP, SH), dtype=np.float32)
        np.add.at(mt, (vrow[sel], dloc[sel]), 1.0)
        assert mt.max() <= 16, "edge multiplicity exceeds exact f8 ints"
        mtl = (mt.reshape(VT, 128, NCHUNK, DB).transpose(1, 2, 0, 3)
               .reshape(128, NCHUNK, 20, 4, DB)
               .reshape(128, NCHUNK * 20 * 4 * DB)).astype(F8)

        in_maps.append(dict(
            hT0=hT0,
            hD0=np.ascontiguousarray(hD0),
            hr0=hr0,
            mt=np.ascontiguousarray(mtl),
            ident=ident,
            betas=betas_rep,
        ))
    return in_maps


def kernel(feats, src, dst, betas):
    import time

    in_maps = _make_in_maps(feats, src, dst, betas)
    nc = _get_kernel()
    res = None
    last_err = None
    for attempt in range(3):
        try:
            res = _run(nc, in_maps)
            break
        except Exception as e:  # transient device/tunnel hiccups observed
            last_err = e
            time.sleep(20)
    if res is None:
        raise last_err
    shards = [np.asarray(res.results[c]["out"], dtype=np.float32)
              for c in range(NCORES)]
    return np.concatenate(shards, axis=0)
